# revision 1
# baseline (speedup 1.0000x reference)
"""2-layer GCN (GCNConv -> ReLU -> GCNConv -> ReLU -> two linear heads) on 8
Trainium2 NeuronCores.

Strategy:
  - Destination nodes sharded across 8 cores (12500 each).
  - Aggregation (symmetric-normalized scatter-add incl. self-loops) is done as
    gather + segment-matmul: for each 128-edge tile, dma_gather pulls the
    source rows (256B each) from the feature table in HBM; the DVE builds a
    [128 edges x BLOCK dst] fp16 indicator (is_equal vs iota, scaled by the
    edge norm); the PE multiplies gathered-rows^T @ indicator (fp16 inputs,
    fp32 PSUM accumulation), accumulating the per-block aggregate in PSUM.
  - Feature tables are fp16 rows padded to 256B (64 fp16 + 64 pad) so the
    gather granularity constraint (256B) is met without extra traffic and the
    PE runs at 1 cycle/row instead of fp32's 4.
  - Layer 1 aggregates x then applies W1 (fp32). Layer 2 applies W2 locally
    (fp32), AllGathers the per-core fp16 p2 shards, then aggregates and
    applies the two 64->1 heads (fused as one 64->2 fp32 matmul).
  - int16 gather indices limit one gather to 32768 table rows, so sources are
    chunked 4x (25000 rows per chunk).
"""

import math
import os
import numpy as np

N_NODES = 100000
IN_D = 64
HID = 128
OUT_D = 64
N_CORES = 8
PER_CORE = N_NODES // N_CORES  # 12500
BLOCK = 256  # dst nodes per block (PSUM accumulator width)
N_CHUNKS = 4
CHUNK = 25000  # source rows per gather chunk (< 32768 for int16 idxs)
GROUP = 1  # blocks per gather instruction
P = 128
ROW = 128  # fp16 elements per padded table row (= 256B)


# ----------------------------------------------------------------------------
# Host-side preprocessing
# ----------------------------------------------------------------------------

def preprocess(edge_index):
    """Bucket edges (plus self-loops) by (core, block, chunk); build per-core
    int16 gather-index images and per-tile (dstloc, norm) fp16 metadata
    images.
    """
    src = np.asarray(edge_index[0], dtype=np.int64)
    dst = np.asarray(edge_index[1], dtype=np.int64)

    deg = np.bincount(dst, minlength=N_NODES).astype(np.float32) + 1.0
    dinv = (1.0 / np.sqrt(deg)).astype(np.float32)

    loops = np.arange(N_NODES, dtype=np.int64)
    s_all = np.concatenate([src, loops])
    d_all = np.concatenate([dst, loops])
    norm_all = dinv[s_all] * dinv[d_all]

    core = d_all // PER_CORE
    dst_local = d_all - core * PER_CORE
    block = dst_local // BLOCK
    chunk = s_all // CHUNK
    idx_local = (s_all - chunk * CHUNK).astype(np.int16)
    dstloc = (dst_local - block * BLOCK).astype(np.float32)

    nblk = math.ceil(PER_CORE / BLOCK)
    bucket = ((core * nblk) + block) * N_CHUNKS + chunk
    n_buckets = N_CORES * nblk * N_CHUNKS
    counts = np.bincount(bucket, minlength=n_buckets)
    counts3 = counts.reshape(N_CORES, nblk, N_CHUNKS)
    # per-(block, chunk) tile budget: max over cores only (SPMD needs
    # core-uniformity, not block-uniformity)
    T_bc = np.ceil(counts3.max(axis=0) / P).astype(np.int64)  # [nblk, chunks]
    T_bc = np.maximum(T_bc, 1)
    budgets = T_bc * P  # [nblk, chunks]
    # per-core flat stream: buckets in (block, chunk) order, each padded to
    # its own budget
    off_bc = np.zeros(nblk * N_CHUNKS + 1, np.int64)
    np.cumsum(budgets.reshape(-1), out=off_bc[1:])
    S = int(off_bc[-1])  # slots per core

    order = np.argsort(bucket, kind="stable")
    b_sorted = bucket[order]
    start_of = np.zeros(n_buckets + 1, np.int64)
    np.cumsum(counts, out=start_of[1:])
    pos_in_bucket = np.arange(len(order)) - start_of[b_sorted]
    core_of = b_sorted // (nblk * N_CHUNKS)
    local_bc = b_sorted % (nblk * N_CHUNKS)
    slot = core_of * S + off_bc[local_bc] + pos_in_bucket

    total_slots = N_CORES * S
    idx_flat = np.zeros(total_slots, np.int16)
    nrm_flat = np.zeros(total_slots, np.float32)
    dl_flat = np.zeros(total_slots, np.float32)
    idx_flat[slot] = idx_local[order]
    nrm_flat[slot] = norm_all[order]
    dl_flat[slot] = dstloc[order]
    idx_flat = idx_flat.reshape(N_CORES, S)
    nrm_flat = nrm_flat.reshape(N_CORES, S)
    dl_flat = dl_flat.reshape(N_CORES, S)

    assert GROUP == 1, "per-bucket budgets assume GROUP == 1"
    per_core = []
    for c in range(N_CORES):
        idx_img = np.tile(idx_flat[c].reshape(-1, 16).T, (8, 1))
        per_core.append({
            "idx": np.ascontiguousarray(idx_img),
            "nrm": np.ascontiguousarray(nrm_flat[c].reshape(-1, P).T),
            "dl": np.ascontiguousarray(dl_flat[c].reshape(-1, P).T),
        })
    return [list(map(int, r)) for r in T_bc], per_core


# ----------------------------------------------------------------------------
# Device program
# ----------------------------------------------------------------------------

def build_program(T, n_devices=N_CORES, collective=True):
    import concourse.bacc as bacc
    import concourse.mybir as mybir
    import concourse.tile as tile
    from concourse.masks import make_identity
    from contextlib import ExitStack

    f32 = mybir.dt.float32
    f16 = mybir.dt.float16
    AF = mybir.ActivationFunctionType

    nblk = math.ceil(PER_CORE / BLOCK)
    T_bc = T  # [nblk][N_CHUNKS] per-bucket tile counts
    assert len(T_bc) == nblk
    n_tiles_core = sum(sum(r) for r in T_bc)
    idx_cols = n_tiles_core * 8
    # tile-column offset of each (block, chunk) bucket
    toff = [[0] * N_CHUNKS for _ in range(nblk)]
    acc = 0
    for b in range(nblk):
        for ch in range(N_CHUNKS):
            toff[b][ch] = acc
            acc += T_bc[b][ch]

    nc = bacc.Bacc("TRN2", target_bir_lowering=False, debug=False,
                   num_devices=n_devices)

    x_d = nc.dram_tensor("x16", [N_NODES, ROW], f16, kind="ExternalInput").ap()
    idx_d = nc.dram_tensor("idx_img", [P, idx_cols], mybir.dt.int16,
                           kind="ExternalInput").ap()
    nrm_d = nc.dram_tensor("nrm_img", [P, n_tiles_core], f32,
                           kind="ExternalInput").ap()
    dl_d = nc.dram_tensor("dl_img", [P, n_tiles_core], f32,
                          kind="ExternalInput").ap()
    iota_d = nc.dram_tensor("iota", [P, BLOCK], f16, kind="ExternalInput").ap()
    w1_d = nc.dram_tensor("W1", [IN_D, HID], f32, kind="ExternalInput").ap()
    b1_d = nc.dram_tensor("b1", [HID, 1], f32, kind="ExternalInput").ap()
    w2_d = nc.dram_tensor("W2", [HID, OUT_D], f32, kind="ExternalInput").ap()
    b2_d = nc.dram_tensor("b2", [OUT_D, 1], f32, kind="ExternalInput").ap()
    wh_d = nc.dram_tensor("Wh", [OUT_D, 2], f32, kind="ExternalInput").ap()
    bh_d = nc.dram_tensor("bh", [2, 1], f32, kind="ExternalInput").ap()
    out_d = nc.dram_tensor("out", [2, PER_CORE], f32, kind="ExternalOutput").ap()

    with tile.TileContext(nc) as tc, ExitStack() as es:
        consts = es.enter_context(tc.tile_pool(name="consts", bufs=1))
        dram = es.enter_context(tc.tile_pool(name="dram", bufs=1, space="DRAM"))
        p_g = es.enter_context(tc.tile_pool(name="p_g", bufs=int(os.environ.get("GCN_GBUFS", "6"))))
        p_ind = es.enter_context(tc.tile_pool(name="p_ind", bufs=8))
        p_sb = es.enter_context(tc.tile_pool(name="p_sb", bufs=4))
        p_out = es.enter_context(tc.tile_pool(name="p_out", bufs=3))
        p_ps_agg = es.enter_context(tc.tile_pool(name="ps_agg", bufs=4, space="PSUM"))
        p_ps_h = es.enter_context(tc.tile_pool(name="ps_h", bufs=2, space="PSUM"))
        p_ps_t = es.enter_context(tc.tile_pool(name="ps_t", bufs=2, space="PSUM"))

        iota_s = consts.tile([P, BLOCK], f16)
        nc.sync.dma_start(iota_s[:], iota_d[:])
        w1_s = consts.tile([IN_D, HID], f32)
        nc.sync.dma_start(w1_s[:], w1_d[:])
        b1_s = consts.tile([HID, 1], f32)
        nc.sync.dma_start(b1_s[:], b1_d[:])
        w2_s = consts.tile([HID, OUT_D], f32)
        nc.sync.dma_start(w2_s[:], w2_d[:])
        b2_s = consts.tile([OUT_D, 1], f32)
        nc.sync.dma_start(b2_s[:], b2_d[:])
        wh_s = consts.tile([OUT_D, 2], f32)
        nc.sync.dma_start(wh_s[:], wh_d[:])
        bh_s = consts.tile([2, 1], f32)
        nc.sync.dma_start(bh_s[:], bh_d[:])
        zero_s = consts.tile([1, BLOCK], f16)
        nc.vector.memset(zero_s[:], 0.0)
        ident_s = consts.tile([P, P], f32)
        make_identity(nc, ident_s[:])

        idx_all = consts.tile([P, idx_cols], mybir.dt.int16)
        nc.sync.dma_start(idx_all[:], idx_d[:])
        nrm_all = consts.tile([P, n_tiles_core], f32)
        nc.sync.dma_start(nrm_all[:], nrm_d[:])
        dl_all = consts.tile([P, n_tiles_core], f32)
        nc.sync.dma_start(dl_all[:], dl_d[:])

        p2_loc = dram.tile([PER_CORE, ROW], f16)
        if collective:
            p2_full = dram.tile([N_NODES, ROW], f16, addr_space="Shared")
        else:
            p2_full = dram.tile([N_NODES, ROW], f16)

        def aggregate_layer(table, epilogue):
            for b in range(nblk):
                g_tiles = []
                for ch in range(N_CHUNKS):
                    tch = T_bc[b][ch]
                    ioff = toff[b][ch] * 8
                    ni = tch * P
                    gt = p_g.tile([P, tch, ROW], f16, tag=f"g{ch}")
                    nc.gpsimd.dma_gather(
                        gt[:], table[ch * CHUNK:(ch + 1) * CHUNK, :],
                        idx_all[:, ioff:ioff + tch * 8],
                        num_idxs=ni, num_idxs_reg=ni, elem_size=ROW,
                        single_packet=False,
                    )
                    g_tiles.append(gt)

                psum = p_ps_agg.tile([IN_D, BLOCK], f32, tag="agg")
                n_mm = sum(T_bc[b])
                k = 0
                for ch in range(N_CHUNKS):
                    for t in range(T_bc[b][ch]):
                        col = toff[b][ch] + t
                        ind = p_ind.tile([P, BLOCK], f16, tag="ind")
                        nc.vector.tensor_scalar(
                            ind[:], iota_s[:],
                            dl_all[:, col:col + 1], nrm_all[:, col:col + 1],
                            op0=mybir.AluOpType.is_equal,
                            op1=mybir.AluOpType.mult,
                        )
                        k += 1
                        nc.tensor.matmul(
                            psum[:], g_tiles[ch][:, t, :IN_D],
                            ind[:], start=(k == 1), stop=(k == n_mm),
                        )
                epilogue(b, psum)

        # ---------------- layer 1 ----------------
        def epi1(b, psum):
            aggT = p_sb.tile([IN_D, BLOCK], f32, tag="aggT")
            nc.scalar.activation(aggT[:], psum[:], AF.Copy)
            ps_h = p_ps_h.tile([HID, BLOCK], f32, tag="mm")
            nc.tensor.matmul(ps_h[:], w1_s[:], aggT[:], start=True, stop=True)
            h1 = p_sb.tile([HID, BLOCK], f32, tag="h1")
            nc.scalar.activation(h1[:], ps_h[:], AF.Relu, bias=b1_s[:, :1])
            ps_p_full = p_ps_h.tile([HID, BLOCK], f32, tag="mm")
            ps_p = ps_p_full[:OUT_D]
            nc.tensor.matmul(ps_p, w2_s[:], h1[:], start=True, stop=True)
            p2T = p_sb.tile([OUT_D, BLOCK], f32, tag="p2T")
            nc.scalar.activation(p2T[:], ps_p, AF.Copy)
            tp = min(P, BLOCK)
            for h in range(max(1, BLOCK // P)):
                rows0 = b * BLOCK + h * tp
                nrows = min(tp, PER_CORE - rows0)
                if nrows <= 0:
                    continue
                ps_t_full = p_ps_t.tile([P, max(BLOCK, OUT_D)], f32, tag="small")
                ps_t = ps_t_full[:tp, :OUT_D]
                nc.tensor.transpose(ps_t, p2T[:, h * tp:(h + 1) * tp],
                                    identity=ident_s[:OUT_D, :OUT_D])
                p2s = p_sb.tile([tp, OUT_D], f16, tag="p2s")
                nc.scalar.activation(p2s[:], ps_t, AF.Copy)
                nc.sync.dma_start(p2_loc[rows0:rows0 + nrows, :OUT_D],
                                  p2s[:nrows, :])

        aggregate_layer(x_d, epi1)

        if collective:
            nc.gpsimd.collective_compute(
                "AllGather", mybir.AluOpType.bypass,
                ins=[p2_loc.opt()], outs=[p2_full.opt()],
                replica_groups=[list(range(N_CORES))],
            )
        else:
            nc.sync.dma_start(p2_full[:PER_CORE, :].opt(), p2_loc[:].opt())

        # ---------------- layer 2 + heads ----------------
        def epi2(b, psum):
            h2 = p_sb.tile([OUT_D, BLOCK], f32, tag="h2")
            nc.scalar.activation(h2[:], psum[:], AF.Relu, bias=b2_s[:, :1])
            ps_o_full = p_ps_t.tile([P, max(BLOCK, OUT_D)], f32, tag="small")
            ps_o = ps_o_full[:2, :BLOCK]
            nc.tensor.matmul(ps_o, wh_s[:], h2[:], start=True, stop=True)
            ob = p_out.tile([2, BLOCK], f32, tag="ob")
            nc.vector.tensor_scalar_add(ob[:], ps_o, bh_s[:, :1])
            ncols = min(BLOCK, PER_CORE - b * BLOCK)
            nc.sync.dma_start(out_d[:, b * BLOCK:b * BLOCK + ncols],
                              ob[:, :ncols])

        aggregate_layer(p2_full.opt(), epi2)

    nc.compile()
    return nc


# ----------------------------------------------------------------------------
# Entry point
# ----------------------------------------------------------------------------

def make_in_maps(inputs, per_core):
    x = np.asarray(inputs["x"], dtype=np.float32)
    x16 = np.zeros((N_NODES, ROW), np.float16)
    x16[:, :IN_D] = x.astype(np.float16)
    iota = np.broadcast_to(np.arange(BLOCK, dtype=np.float16), (P, BLOCK))
    wh = np.concatenate([np.asarray(inputs["Wd"], np.float32),
                         np.asarray(inputs["Wp"], np.float32)], axis=1)
    bh = np.array([[np.float32(np.asarray(inputs["bd"]).reshape(-1)[0])],
                   [np.float32(np.asarray(inputs["bp"]).reshape(-1)[0])]],
                  np.float32)
    in_maps = []
    for c in range(N_CORES):
        in_maps.append({
            "x16": x16,
            "idx_img": per_core[c]["idx"],
            "nrm_img": per_core[c]["nrm"],
            "dl_img": per_core[c]["dl"],
            "iota": np.ascontiguousarray(iota),
            "W1": np.ascontiguousarray(np.asarray(inputs["W1"], np.float32)),
            "b1": np.asarray(inputs["b1"], np.float32).reshape(HID, 1),
            "W2": np.ascontiguousarray(np.asarray(inputs["W2"], np.float32)),
            "b2": np.asarray(inputs["b2"], np.float32).reshape(OUT_D, 1),
            "Wh": np.ascontiguousarray(wh),
            "bh": bh,
        })
    return in_maps


def kernel(x, edge_index, W1, b1, W2, b2, Wd, bd, Wp, bp):
    from concourse import bass_utils

    T, per_core = preprocess(edge_index)
    nc = build_program(T)
    in_maps = make_in_maps(dict(x=x, W1=W1, b1=b1, W2=W2, b2=b2, Wd=Wd,
                                bd=bd, Wp=Wp, bp=bp), per_core)
    res = bass_utils.run_bass_kernel_spmd(nc, in_maps,
                                          core_ids=list(range(N_CORES)))
    dur = np.empty((N_NODES, 1), np.float32)
    pha = np.empty((N_NODES, 1), np.float32)
    for c in range(N_CORES):
        o = res.results[c]["out"]
        dur[c * PER_CORE:(c + 1) * PER_CORE, 0] = o[0]
        pha[c * PER_CORE:(c + 1) * PER_CORE, 0] = o[1]
    return dur, pha



# revision 4
# speedup vs baseline: 1552.5138x; 1552.5138x over previous
"""2-layer GCN (GCNConv -> ReLU -> GCNConv -> ReLU -> two linear heads) on 8
Trainium2 NeuronCores.

Strategy:
  - Destination nodes sharded across 8 cores (12500 each).
  - Aggregation (symmetric-normalized scatter-add incl. self-loops) is done as
    gather + segment-matmul: for each 128-edge tile, dma_gather pulls the
    source rows (256B each) from the feature table in HBM; the DVE builds a
    [128 edges x BLOCK dst] fp16 indicator (is_equal vs iota, scaled by the
    edge norm); the PE multiplies gathered-rows^T @ indicator (fp16 inputs,
    fp32 PSUM accumulation), accumulating the per-block aggregate in PSUM.
  - Feature tables are fp16 rows padded to 256B (64 fp16 + 64 pad) so the
    gather granularity constraint (256B) is met without extra traffic and the
    PE runs at 1 cycle/row instead of fp32's 4.
  - Layer 1 aggregates x then applies W1 (fp32). Layer 2 applies W2 locally
    (fp32), AllGathers the per-core fp16 p2 shards, then aggregates and
    applies the two 64->1 heads (fused as one 64->2 fp32 matmul).
  - int16 gather indices limit one gather to 32768 table rows, so sources are
    chunked 4x (25000 rows per chunk).
"""

import math
import os
import numpy as np

N_NODES = 100000
IN_D = 64
HID = 128
OUT_D = 64
N_CORES = 8
PER_CORE = N_NODES // N_CORES  # 12500
BLOCK = 128  # dst nodes per block (PSUM accumulator width)
N_CHUNKS = 4
CHUNK = 25000  # source rows per gather chunk (< 32768 for int16 idxs)
GROUP = 1  # blocks per gather instruction
P = 128
ROW = 128  # fp16 elements per padded table row (= 256B)


# ----------------------------------------------------------------------------
# Host-side preprocessing
# ----------------------------------------------------------------------------

def preprocess(edge_index):
    """Bucket edges (plus self-loops) by (core, block, chunk); build per-core
    int16 gather-index images and per-tile (dstloc, norm) fp16 metadata
    images.
    """
    src = np.asarray(edge_index[0], dtype=np.int64)
    dst = np.asarray(edge_index[1], dtype=np.int64)

    deg = np.bincount(dst, minlength=N_NODES).astype(np.float32) + 1.0
    dinv = (1.0 / np.sqrt(deg)).astype(np.float32)

    loops = np.arange(N_NODES, dtype=np.int64)
    s_all = np.concatenate([src, loops])
    d_all = np.concatenate([dst, loops])
    norm_all = dinv[s_all] * dinv[d_all]

    core = d_all // PER_CORE
    dst_local = d_all - core * PER_CORE
    block = dst_local // BLOCK
    chunk = s_all // CHUNK
    idx_local = (s_all - chunk * CHUNK).astype(np.int16)
    dstloc = (dst_local - block * BLOCK).astype(np.float32)

    nblk = math.ceil(PER_CORE / BLOCK)
    bucket = ((core * nblk) + block) * N_CHUNKS + chunk
    n_buckets = N_CORES * nblk * N_CHUNKS
    counts = np.bincount(bucket, minlength=n_buckets)
    counts3 = counts.reshape(N_CORES, nblk, N_CHUNKS)
    # per-(block, chunk) tile budget: max over cores only (SPMD needs
    # core-uniformity, not block-uniformity)
    T_bc = np.ceil(counts3.max(axis=0) / P).astype(np.int64)  # [nblk, chunks]
    T_bc = np.maximum(T_bc, 1)
    budgets = T_bc * P  # [nblk, chunks]
    # per-core flat stream: buckets in (block, chunk) order, each padded to
    # its own budget
    off_bc = np.zeros(nblk * N_CHUNKS + 1, np.int64)
    np.cumsum(budgets.reshape(-1), out=off_bc[1:])
    S = int(off_bc[-1])  # slots per core

    order = np.argsort(bucket, kind="stable")
    b_sorted = bucket[order]
    start_of = np.zeros(n_buckets + 1, np.int64)
    np.cumsum(counts, out=start_of[1:])
    pos_in_bucket = np.arange(len(order)) - start_of[b_sorted]
    core_of = b_sorted // (nblk * N_CHUNKS)
    local_bc = b_sorted % (nblk * N_CHUNKS)
    slot = core_of * S + off_bc[local_bc] + pos_in_bucket

    total_slots = N_CORES * S
    idx_flat = np.zeros(total_slots, np.int16)
    nrm_flat = np.zeros(total_slots, np.float32)
    dl_flat = np.zeros(total_slots, np.float32)
    idx_flat[slot] = idx_local[order]
    nrm_flat[slot] = norm_all[order]
    dl_flat[slot] = dstloc[order]
    idx_flat = idx_flat.reshape(N_CORES, S)
    nrm_flat = nrm_flat.reshape(N_CORES, S)
    dl_flat = dl_flat.reshape(N_CORES, S)

    assert GROUP == 1, "per-bucket budgets assume GROUP == 1"
    per_core = []
    for c in range(N_CORES):
        idx_img = np.tile(idx_flat[c].reshape(-1, 16).T, (8, 1))
        per_core.append({
            "idx": np.ascontiguousarray(idx_img),
            "nrm": np.ascontiguousarray(nrm_flat[c].reshape(-1, P).T),
            "dl": np.ascontiguousarray(dl_flat[c].reshape(-1, P).T),
        })
    return [list(map(int, r)) for r in T_bc], per_core


# ----------------------------------------------------------------------------
# Device program
# ----------------------------------------------------------------------------

def build_program(T, n_devices=N_CORES, collective=True):
    import concourse.bacc as bacc
    import concourse.mybir as mybir
    import concourse.tile as tile
    from concourse.masks import make_identity
    from contextlib import ExitStack

    f32 = mybir.dt.float32
    f16 = mybir.dt.float16
    AF = mybir.ActivationFunctionType

    nblk = math.ceil(PER_CORE / BLOCK)
    T_bc = T  # [nblk][N_CHUNKS] per-bucket tile counts
    assert len(T_bc) == nblk
    n_tiles_core = sum(sum(r) for r in T_bc)
    idx_cols = n_tiles_core * 8
    # tile-column offset of each (block, chunk) bucket
    toff = [[0] * N_CHUNKS for _ in range(nblk)]
    acc = 0
    for b in range(nblk):
        for ch in range(N_CHUNKS):
            toff[b][ch] = acc
            acc += T_bc[b][ch]

    nc = bacc.Bacc("TRN2", target_bir_lowering=False, debug=False,
                   num_devices=n_devices, num_swdge_queues=4)

    x_d = nc.dram_tensor("x16", [N_NODES, ROW], f16, kind="ExternalInput").ap()
    idx_d = nc.dram_tensor("idx_img", [P, idx_cols], mybir.dt.int16,
                           kind="ExternalInput").ap()
    nrm_d = nc.dram_tensor("nrm_img", [P, n_tiles_core], f32,
                           kind="ExternalInput").ap()
    dl_d = nc.dram_tensor("dl_img", [P, n_tiles_core], f32,
                          kind="ExternalInput").ap()
    iota_d = nc.dram_tensor("iota", [P, BLOCK], f16, kind="ExternalInput").ap()
    w1_d = nc.dram_tensor("W1", [IN_D, HID], f32, kind="ExternalInput").ap()
    b1_d = nc.dram_tensor("b1", [HID, 1], f32, kind="ExternalInput").ap()
    w2_d = nc.dram_tensor("W2", [HID, OUT_D], f32, kind="ExternalInput").ap()
    b2_d = nc.dram_tensor("b2", [OUT_D, 1], f32, kind="ExternalInput").ap()
    wh_d = nc.dram_tensor("Wh", [OUT_D, 2], f32, kind="ExternalInput").ap()
    bh_d = nc.dram_tensor("bh", [2, 1], f32, kind="ExternalInput").ap()
    out_d = nc.dram_tensor("out", [2, PER_CORE], f32, kind="ExternalOutput").ap()

    with tile.TileContext(nc) as tc, ExitStack() as es:
        consts = es.enter_context(tc.tile_pool(name="consts", bufs=1))
        dram = es.enter_context(tc.tile_pool(name="dram", bufs=1, space="DRAM"))
        p_g = es.enter_context(tc.tile_pool(name="p_g", bufs=int(os.environ.get("GCN_GBUFS", "6"))))
        p_ind = es.enter_context(tc.tile_pool(name="p_ind", bufs=8))
        p_sb = es.enter_context(tc.tile_pool(name="p_sb", bufs=4))
        p_out = es.enter_context(tc.tile_pool(name="p_out", bufs=3))
        p_ps_agg = es.enter_context(tc.tile_pool(name="ps_agg", bufs=4, space="PSUM"))
        p_ps_h = es.enter_context(tc.tile_pool(name="ps_h", bufs=2, space="PSUM"))
        p_ps_t = es.enter_context(tc.tile_pool(name="ps_t", bufs=2, space="PSUM"))

        iota_s = consts.tile([P, BLOCK], f16)
        nc.sync.dma_start(iota_s[:], iota_d[:])
        w1_s = consts.tile([IN_D, HID], f32)
        nc.sync.dma_start(w1_s[:], w1_d[:])
        b1_s = consts.tile([HID, 1], f32)
        nc.sync.dma_start(b1_s[:], b1_d[:])
        w2_s = consts.tile([HID, OUT_D], f32)
        nc.sync.dma_start(w2_s[:], w2_d[:])
        b2_s = consts.tile([OUT_D, 1], f32)
        nc.sync.dma_start(b2_s[:], b2_d[:])
        wh_s = consts.tile([OUT_D, 2], f32)
        nc.sync.dma_start(wh_s[:], wh_d[:])
        bh_s = consts.tile([2, 1], f32)
        nc.sync.dma_start(bh_s[:], bh_d[:])
        zero_s = consts.tile([1, BLOCK], f16)
        nc.vector.memset(zero_s[:], 0.0)
        ident_s = consts.tile([P, P], f32)
        make_identity(nc, ident_s[:])

        idx_all = consts.tile([P, idx_cols], mybir.dt.int16)
        nc.sync.dma_start(idx_all[:], idx_d[:])
        nrm_all = consts.tile([P, n_tiles_core], f32)
        nc.sync.dma_start(nrm_all[:], nrm_d[:])
        dl_all = consts.tile([P, n_tiles_core], f32)
        nc.sync.dma_start(dl_all[:], dl_d[:])

        p2_loc = dram.tile([PER_CORE, ROW], f16)
        if collective:
            p2_full = dram.tile([N_NODES, ROW], f16, addr_space="Shared")
        else:
            p2_full = dram.tile([N_NODES, ROW], f16)

        def aggregate_layer(table, epilogue):
            for b in range(nblk):
                g_tiles = []
                for ch in range(N_CHUNKS):
                    tch = T_bc[b][ch]
                    ioff = toff[b][ch] * 8
                    ni = tch * P
                    gt = p_g.tile([P, tch, ROW], f16, tag=f"g{ch}")
                    nc.gpsimd.dma_gather(
                        gt[:], table[ch * CHUNK:(ch + 1) * CHUNK, :],
                        idx_all[:, ioff:ioff + tch * 8],
                        num_idxs=ni, num_idxs_reg=ni, elem_size=ROW,
                        single_packet=False, queue_num=ch,
                    )
                    g_tiles.append(gt)

                psum = p_ps_agg.tile([IN_D, BLOCK], f32, tag="agg")
                n_mm = sum(T_bc[b])
                k = 0
                for ch in range(N_CHUNKS):
                    for t in range(T_bc[b][ch]):
                        col = toff[b][ch] + t
                        ind = p_ind.tile([P, BLOCK], f16, tag="ind")
                        nc.vector.tensor_scalar(
                            ind[:], iota_s[:],
                            dl_all[:, col:col + 1], nrm_all[:, col:col + 1],
                            op0=mybir.AluOpType.is_equal,
                            op1=mybir.AluOpType.mult,
                        )
                        k += 1
                        nc.tensor.matmul(
                            psum[:], g_tiles[ch][:, t, :IN_D],
                            ind[:], start=(k == 1), stop=(k == n_mm),
                        )
                epilogue(b, psum)

        # ---------------- layer 1 ----------------
        def epi1(b, psum):
            aggT = p_sb.tile([IN_D, BLOCK], f32, tag="aggT")
            nc.scalar.activation(aggT[:], psum[:], AF.Copy)
            ps_h = p_ps_h.tile([HID, BLOCK], f32, tag="mm")
            nc.tensor.matmul(ps_h[:], w1_s[:], aggT[:], start=True, stop=True)
            h1 = p_sb.tile([HID, BLOCK], f32, tag="h1")
            nc.scalar.activation(h1[:], ps_h[:], AF.Relu, bias=b1_s[:, :1])
            ps_p_full = p_ps_h.tile([HID, BLOCK], f32, tag="mm")
            ps_p = ps_p_full[:OUT_D]
            nc.tensor.matmul(ps_p, w2_s[:], h1[:], start=True, stop=True)
            p2T = p_sb.tile([OUT_D, BLOCK], f32, tag="p2T")
            nc.scalar.activation(p2T[:], ps_p, AF.Copy)
            tp = min(P, BLOCK)
            for h in range(max(1, BLOCK // P)):
                rows0 = b * BLOCK + h * tp
                nrows = min(tp, PER_CORE - rows0)
                if nrows <= 0:
                    continue
                ps_t_full = p_ps_t.tile([P, max(BLOCK, OUT_D)], f32, tag="small")
                ps_t = ps_t_full[:tp, :OUT_D]
                nc.tensor.transpose(ps_t, p2T[:, h * tp:(h + 1) * tp],
                                    identity=ident_s[:OUT_D, :OUT_D])
                p2s = p_sb.tile([tp, OUT_D], f16, tag="p2s")
                nc.scalar.activation(p2s[:], ps_t, AF.Copy)
                nc.sync.dma_start(p2_loc[rows0:rows0 + nrows, :OUT_D],
                                  p2s[:nrows, :])

        aggregate_layer(x_d, epi1)

        if collective:
            nc.gpsimd.collective_compute(
                "AllGather", mybir.AluOpType.bypass,
                ins=[p2_loc.opt()], outs=[p2_full.opt()],
                replica_groups=[list(range(N_CORES))],
            )
        else:
            nc.sync.dma_start(p2_full[:PER_CORE, :].opt(), p2_loc[:].opt())

        # ---------------- layer 2 + heads ----------------
        def epi2(b, psum):
            h2 = p_sb.tile([OUT_D, BLOCK], f32, tag="h2")
            nc.scalar.activation(h2[:], psum[:], AF.Relu, bias=b2_s[:, :1])
            ps_o_full = p_ps_t.tile([P, max(BLOCK, OUT_D)], f32, tag="small")
            ps_o = ps_o_full[:2, :BLOCK]
            nc.tensor.matmul(ps_o, wh_s[:], h2[:], start=True, stop=True)
            ob = p_out.tile([2, BLOCK], f32, tag="ob")
            nc.vector.tensor_scalar_add(ob[:], ps_o, bh_s[:, :1])
            ncols = min(BLOCK, PER_CORE - b * BLOCK)
            nc.sync.dma_start(out_d[:, b * BLOCK:b * BLOCK + ncols],
                              ob[:, :ncols])

        aggregate_layer(p2_full.opt(), epi2)

    nc.compile()
    return nc


# ----------------------------------------------------------------------------
# Entry point
# ----------------------------------------------------------------------------

def make_in_maps(inputs, per_core):
    x = np.asarray(inputs["x"], dtype=np.float32)
    x16 = np.zeros((N_NODES, ROW), np.float16)
    x16[:, :IN_D] = x.astype(np.float16)
    iota = np.broadcast_to(np.arange(BLOCK, dtype=np.float16), (P, BLOCK))
    wh = np.concatenate([np.asarray(inputs["Wd"], np.float32),
                         np.asarray(inputs["Wp"], np.float32)], axis=1)
    bh = np.array([[np.float32(np.asarray(inputs["bd"]).reshape(-1)[0])],
                   [np.float32(np.asarray(inputs["bp"]).reshape(-1)[0])]],
                  np.float32)
    in_maps = []
    for c in range(N_CORES):
        in_maps.append({
            "x16": x16,
            "idx_img": per_core[c]["idx"],
            "nrm_img": per_core[c]["nrm"],
            "dl_img": per_core[c]["dl"],
            "iota": np.ascontiguousarray(iota),
            "W1": np.ascontiguousarray(np.asarray(inputs["W1"], np.float32)),
            "b1": np.asarray(inputs["b1"], np.float32).reshape(HID, 1),
            "W2": np.ascontiguousarray(np.asarray(inputs["W2"], np.float32)),
            "b2": np.asarray(inputs["b2"], np.float32).reshape(OUT_D, 1),
            "Wh": np.ascontiguousarray(wh),
            "bh": bh,
        })
    return in_maps


def kernel(x, edge_index, W1, b1, W2, b2, Wd, bd, Wp, bp):
    from concourse import bass_utils

    T, per_core = preprocess(edge_index)
    nc = build_program(T)
    in_maps = make_in_maps(dict(x=x, W1=W1, b1=b1, W2=W2, b2=b2, Wd=Wd,
                                bd=bd, Wp=Wp, bp=bp), per_core)
    res = bass_utils.run_bass_kernel_spmd(nc, in_maps,
                                          core_ids=list(range(N_CORES)))
    dur = np.empty((N_NODES, 1), np.float32)
    pha = np.empty((N_NODES, 1), np.float32)
    for c in range(N_CORES):
        o = res.results[c]["out"]
        dur[c * PER_CORE:(c + 1) * PER_CORE, 0] = o[0]
        pha[c * PER_CORE:(c + 1) * PER_CORE, 0] = o[1]
    return dur, pha



# revision 12
# speedup vs baseline: 2933.3963x; 1.8894x over previous
"""2-layer GCN (GCNConv -> ReLU -> GCNConv -> ReLU -> two linear heads) on 8
Trainium2 NeuronCores.

Strategy:
  - Destination nodes sharded across 8 cores (12500 each).
  - Aggregation (symmetric-normalized scatter-add incl. self-loops) is done as
    gather + segment-matmul: for each 128-edge tile, dma_gather pulls the
    source rows (256B each) from the feature table in HBM; the DVE builds a
    [128 edges x BLOCK dst] fp16 0/1 indicator (is_equal vs iota); the PE
    multiplies gathered-rows^T @ indicator (fp16 inputs, fp32 PSUM
    accumulation), accumulating the per-block aggregate in PSUM.
  - The deg^{-1/2} normalization is split: dinv[src] is pre-multiplied into
    the feature tables (host-side for x, on-device for the layer-2 table);
    dinv[dst] is applied per block as a free-dim column scale on the DVE.
    This keeps the hot loop free of tensor_scalar ops - tensor_scalar can
    enter the DVE's 2-port perf mode, which locks GPSIMD out of SBUF and
    starves SWDGE gather descriptor generation (a ~4x whole-kernel hit).
  - Gathers are spread across 4 SWDGE queues (queue per source-chunk), which
    parallelizes descriptor processing ~4x over a single queue.
  - Feature tables are fp16 rows padded to 256B (64 fp16 + 64 pad) so the
    gather granularity constraint (256B) is met; fp16 also runs the PE at
    full rate.
  - Layer 1 aggregates x~ = dinv*x then applies W1. Layer 2 applies W2
    locally, AllGathers the per-core fp16 p2~ shards, then aggregates and
    applies the two 64->1 heads (fused as one 64->2 fp32 matmul).
  - int16 gather indices limit one gather to 32768 table rows, so sources are
    chunked 4x (25000 rows per chunk).
"""

import math
import os
import numpy as np

N_NODES = 100000
IN_D = 64
HID = 128
OUT_D = 64
N_CORES = 8
PER_CORE = N_NODES // N_CORES  # 12500
BLOCK = 128  # dst nodes per block (PSUM accumulator width)
N_CHUNKS = 4
CHUNK = 25000  # source rows per gather chunk (< 32768 for int16 idxs)
P = 128
ROW = 128  # fp16 elements per padded table row (= 256B)
PAD_DL = 999.0  # dstloc sentinel for pad slots: never matches iota 0..127


# ----------------------------------------------------------------------------
# Host-side preprocessing
# ----------------------------------------------------------------------------

def preprocess(edge_index):
    """Bucket edges (plus self-loops) by (core, block, chunk); build per-core
    int16 gather-index images, per-tile dstloc fp16 images, and per-slot
    dinv images."""
    src = np.asarray(edge_index[0], dtype=np.int64)
    dst = np.asarray(edge_index[1], dtype=np.int64)

    deg = np.bincount(dst, minlength=N_NODES).astype(np.float32) + 1.0
    dinv = (1.0 / np.sqrt(deg)).astype(np.float32)

    loops = np.arange(N_NODES, dtype=np.int64)
    s_all = np.concatenate([src, loops])
    d_all = np.concatenate([dst, loops])

    core = d_all // PER_CORE
    dst_local = d_all - core * PER_CORE
    block = dst_local // BLOCK
    chunk = s_all // CHUNK
    idx_local = (s_all - chunk * CHUNK).astype(np.int16)
    dstloc = (dst_local - block * BLOCK).astype(np.float16)

    nblk = math.ceil(PER_CORE / BLOCK)
    bucket = ((core * nblk) + block) * N_CHUNKS + chunk
    n_buckets = N_CORES * nblk * N_CHUNKS
    counts = np.bincount(bucket, minlength=n_buckets)
    counts3 = counts.reshape(N_CORES, nblk, N_CHUNKS)
    # per-(block, chunk) tile budget: max over cores only (SPMD needs
    # core-uniformity, not block-uniformity)
    T_bc = np.ceil(counts3.max(axis=0) / P).astype(np.int64)  # [nblk, chunks]
    T_bc = np.maximum(T_bc, 1)
    budgets = T_bc * P  # [nblk, chunks]
    off_bc = np.zeros(nblk * N_CHUNKS + 1, np.int64)
    np.cumsum(budgets.reshape(-1), out=off_bc[1:])
    S = int(off_bc[-1])  # slots per core

    order = np.argsort(bucket, kind="stable")
    b_sorted = bucket[order]
    start_of = np.zeros(n_buckets + 1, np.int64)
    np.cumsum(counts, out=start_of[1:])
    pos_in_bucket = np.arange(len(order)) - start_of[b_sorted]
    core_of = b_sorted // (nblk * N_CHUNKS)
    local_bc = b_sorted % (nblk * N_CHUNKS)
    slot = core_of * S + off_bc[local_bc] + pos_in_bucket

    total_slots = N_CORES * S
    idx_flat = np.zeros(total_slots, np.int16)
    dl_flat = np.full(total_slots, PAD_DL, np.float16)
    idx_flat[slot] = idx_local[order]
    dl_flat[slot] = dstloc[order]
    idx_flat = idx_flat.reshape(N_CORES, S)
    dl_flat = dl_flat.reshape(N_CORES, S)

    # per-slot dinv (dst normalization), padded to nblk*BLOCK per core and
    # replicated to 64 partitions (DVE inputs can't partition-broadcast)
    dinv_img = np.zeros((N_CORES, nblk * BLOCK), np.float32)
    dinv_img[:, :PER_CORE] = dinv.reshape(N_CORES, PER_CORE)

    per_core = []
    for c in range(N_CORES):
        idx_img = np.tile(idx_flat[c].reshape(-1, 16).T, (8, 1))
        per_core.append({
            "idx": np.ascontiguousarray(idx_img),
            "dl": np.ascontiguousarray(dl_flat[c].reshape(-1, P).T),
            "dinv": np.ascontiguousarray(
                np.broadcast_to(dinv_img[c:c + 1], (IN_D, nblk * BLOCK))),
        })
    return [list(map(int, r)) for r in T_bc], per_core, dinv


# ----------------------------------------------------------------------------
# Device program
# ----------------------------------------------------------------------------

def build_program(T, n_devices=N_CORES, collective=True):
    import concourse.bacc as bacc
    import concourse.mybir as mybir
    import concourse.tile as tile
    from concourse.masks import make_identity
    from contextlib import ExitStack

    f32 = mybir.dt.float32
    f16 = mybir.dt.float16
    AF = mybir.ActivationFunctionType
    MUL = mybir.AluOpType.mult

    nblk = math.ceil(PER_CORE / BLOCK)
    T_bc = T  # [nblk][N_CHUNKS] per-bucket tile counts
    assert len(T_bc) == nblk
    n_tiles_core = sum(sum(r) for r in T_bc)
    idx_cols = n_tiles_core * 8
    toff = [[0] * N_CHUNKS for _ in range(nblk)]
    acc = 0
    for b in range(nblk):
        for ch in range(N_CHUNKS):
            toff[b][ch] = acc
            acc += T_bc[b][ch]

    nc = bacc.Bacc("TRN2", target_bir_lowering=False, debug=False,
                   num_devices=n_devices, num_swdge_queues=4)

    x_d = nc.dram_tensor("x16", [N_NODES, ROW], f16, kind="ExternalInput").ap()
    idx_d = nc.dram_tensor("idx_img", [P, idx_cols], mybir.dt.int16,
                           kind="ExternalInput").ap()
    dl_d = nc.dram_tensor("dl_img", [P, n_tiles_core], f16,
                          kind="ExternalInput").ap()
    dinv_d = nc.dram_tensor("dinv_img", [IN_D, nblk * BLOCK], f32,
                            kind="ExternalInput").ap()
    iota_d = nc.dram_tensor("iota", [P, BLOCK], f16, kind="ExternalInput").ap()
    w1_d = nc.dram_tensor("W1", [IN_D, HID], f32, kind="ExternalInput").ap()
    b1_d = nc.dram_tensor("b1", [HID, 1], f32, kind="ExternalInput").ap()
    w2_d = nc.dram_tensor("W2", [HID, OUT_D], f32, kind="ExternalInput").ap()
    b2_d = nc.dram_tensor("b2", [OUT_D, 1], f32, kind="ExternalInput").ap()
    wh_d = nc.dram_tensor("Wh", [OUT_D, 2], f32, kind="ExternalInput").ap()
    bh_d = nc.dram_tensor("bh", [2, 1], f32, kind="ExternalInput").ap()
    out_d = nc.dram_tensor("out", [2, PER_CORE], f32, kind="ExternalOutput").ap()

    with tile.TileContext(nc) as tc, ExitStack() as es:
        consts = es.enter_context(tc.tile_pool(name="consts", bufs=1))
        dram = es.enter_context(tc.tile_pool(name="dram", bufs=1, space="DRAM"))
        p_g = es.enter_context(tc.tile_pool(name="p_g", bufs=int(os.environ.get("GCN_GBUFS", "6"))))
        p_ind = es.enter_context(tc.tile_pool(name="p_ind", bufs=8))
        p_sb = es.enter_context(tc.tile_pool(name="p_sb", bufs=4))
        p_out = es.enter_context(tc.tile_pool(name="p_out", bufs=3))
        p_ps_agg = es.enter_context(tc.tile_pool(name="ps_agg", bufs=4, space="PSUM"))
        p_ps_h = es.enter_context(tc.tile_pool(name="ps_h", bufs=2, space="PSUM"))
        p_ps_t = es.enter_context(tc.tile_pool(name="ps_t", bufs=2, space="PSUM"))

        iota_s = consts.tile([P, BLOCK], f16)
        nc.sync.dma_start(iota_s[:], iota_d[:])
        w1_s = consts.tile([IN_D, HID], f32)
        nc.sync.dma_start(w1_s[:], w1_d[:])
        b1_s = consts.tile([HID, 1], f32)
        nc.sync.dma_start(b1_s[:], b1_d[:])
        w2_s = consts.tile([HID, OUT_D], f32)
        nc.sync.dma_start(w2_s[:], w2_d[:])
        b2_s = consts.tile([OUT_D, 1], f32)
        nc.sync.dma_start(b2_s[:], b2_d[:])
        wh_s = consts.tile([OUT_D, 2], f32)
        nc.sync.dma_start(wh_s[:], wh_d[:])
        bh_s = consts.tile([2, 1], f32)
        nc.sync.dma_start(bh_s[:], bh_d[:])
        dinv_s = consts.tile([IN_D, nblk * BLOCK], f32)
        nc.sync.dma_start(dinv_s[:], dinv_d[:])
        ident_s = consts.tile([P, P], f32)
        make_identity(nc, ident_s[:])

        idx_all = consts.tile([P, idx_cols], mybir.dt.int16)
        nc.sync.dma_start(idx_all[:], idx_d[:])
        dl_all = consts.tile([P, n_tiles_core], f16)
        nc.sync.dma_start(dl_all[:], dl_d[:])

        p2_loc = dram.tile([PER_CORE, ROW], f16)
        if collective:
            p2_full = dram.tile([N_NODES, ROW], f16, addr_space="Shared")
        else:
            p2_full = dram.tile([N_NODES, ROW], f16)

        def aggregate_layer(table, epilogue):
            for b in range(nblk):
                g_tiles = []
                for ch in range(N_CHUNKS):
                    tch = T_bc[b][ch]
                    ioff = toff[b][ch] * 8
                    ni = tch * P
                    gt = p_g.tile([P, tch, ROW], f16, tag=f"g{ch}")
                    nc.gpsimd.dma_gather(
                        gt[:], table[ch * CHUNK:(ch + 1) * CHUNK, :],
                        idx_all[:, ioff:ioff + tch * 8],
                        num_idxs=ni, num_idxs_reg=ni, elem_size=ROW,
                        single_packet=False, queue_num=ch,
                    )
                    g_tiles.append(gt)

                psum = p_ps_agg.tile([IN_D, BLOCK], f32, tag="agg")
                n_mm = sum(T_bc[b])
                k = 0
                for ch in range(N_CHUNKS):
                    for t in range(T_bc[b][ch]):
                        col = toff[b][ch] + t
                        ind = p_ind.tile([P, BLOCK], f16, tag="ind")
                        nc.vector.tensor_tensor(
                            ind[:], iota_s[:],
                            dl_all[:, col:col + 1].to_broadcast([P, BLOCK]),
                            op=mybir.AluOpType.is_equal,
                        )
                        k += 1
                        nc.tensor.matmul(
                            psum[:], g_tiles[ch][:, t, :IN_D],
                            ind[:], start=(k == 1), stop=(k == n_mm),
                        )
                epilogue(b, psum)

        # ---------------- layer 1 ----------------
        def epi1(b, psum):
            dv = dinv_s[:, b * BLOCK:(b + 1) * BLOCK]
            # PSUM -> SBUF copy fused with the dinv[dst] column scale (DVE,
            # 1-port tensor_tensor - keeps GPSIMD unblocked)
            aggT = p_sb.tile([IN_D, BLOCK], f32, tag="aggT")
            nc.vector.tensor_tensor(aggT[:], psum[:], dv, op=MUL)
            ps_h = p_ps_h.tile([HID, BLOCK], f32, tag="mm")
            nc.tensor.matmul(ps_h[:], w1_s[:], aggT[:], start=True, stop=True)
            h1 = p_sb.tile([HID, BLOCK], f32, tag="h1")
            nc.scalar.activation(h1[:], ps_h[:], AF.Relu, bias=b1_s[:, :1])
            ps_p_full = p_ps_h.tile([HID, BLOCK], f32, tag="mm")
            ps_p = ps_p_full[:OUT_D]
            nc.tensor.matmul(ps_p, w2_s[:], h1[:], start=True, stop=True)
            # p2~ = dinv[node] * p2 (pre-scale for layer 2's aggregation)
            p2T = p_sb.tile([OUT_D, BLOCK], f32, tag="p2T")
            nc.vector.tensor_tensor(p2T[:], ps_p, dv[:OUT_D], op=MUL)
            rows0 = b * BLOCK
            nrows = min(BLOCK, PER_CORE - rows0)
            ps_t_full = p_ps_t.tile([P, max(BLOCK, OUT_D)], f32, tag="small")
            ps_t = ps_t_full[:BLOCK, :OUT_D]
            nc.tensor.transpose(ps_t, p2T[:, :],
                                identity=ident_s[:OUT_D, :OUT_D])
            p2s = p_sb.tile([BLOCK, OUT_D], f16, tag="p2s")
            nc.scalar.activation(p2s[:], ps_t, AF.Copy)
            nc.sync.dma_start(p2_loc[rows0:rows0 + nrows, :OUT_D],
                              p2s[:nrows, :])

        aggregate_layer(x_d, epi1)

        if collective:
            nc.gpsimd.collective_compute(
                "AllGather", mybir.AluOpType.bypass,
                ins=[p2_loc.opt()], outs=[p2_full.opt()],
                replica_groups=[list(range(N_CORES))],
            )
        else:
            nc.sync.dma_start(p2_full[:PER_CORE, :].opt(), p2_loc[:].opt())

        # ---------------- layer 2 + heads ----------------
        def epi2(b, psum):
            dv = dinv_s[:OUT_D, b * BLOCK:(b + 1) * BLOCK]
            agg2 = p_sb.tile([OUT_D, BLOCK], f32, tag="agg2")
            nc.vector.tensor_tensor(agg2[:], psum[:], dv, op=MUL)
            h2 = p_sb.tile([OUT_D, BLOCK], f32, tag="h2")
            nc.scalar.activation(h2[:], agg2[:], AF.Relu, bias=b2_s[:, :1])
            ps_o_full = p_ps_t.tile([P, max(BLOCK, OUT_D)], f32, tag="small")
            ps_o = ps_o_full[:2, :BLOCK]
            nc.tensor.matmul(ps_o, wh_s[:], h2[:], start=True, stop=True)
            ob = p_out.tile([2, BLOCK], f32, tag="ob")
            nc.vector.tensor_tensor(ob[:], ps_o,
                                    bh_s[:, :1].to_broadcast([2, BLOCK]),
                                    op=mybir.AluOpType.add)
            ncols = min(BLOCK, PER_CORE - b * BLOCK)
            nc.sync.dma_start(out_d[:, b * BLOCK:b * BLOCK + ncols],
                              ob[:, :ncols])

        aggregate_layer(p2_full.opt(), epi2)

    nc.compile()
    return nc


# ----------------------------------------------------------------------------
# Entry point
# ----------------------------------------------------------------------------

def make_in_maps(inputs, per_core, dinv):
    x = np.asarray(inputs["x"], dtype=np.float32)
    x16 = np.zeros((N_NODES, ROW), np.float16)
    x16[:, :IN_D] = (x * dinv[:, None]).astype(np.float16)
    iota = np.broadcast_to(np.arange(BLOCK, dtype=np.float16), (P, BLOCK))
    wh = np.concatenate([np.asarray(inputs["Wd"], np.float32),
                         np.asarray(inputs["Wp"], np.float32)], axis=1)
    bh = np.array([[np.float32(np.asarray(inputs["bd"]).reshape(-1)[0])],
                   [np.float32(np.asarray(inputs["bp"]).reshape(-1)[0])]],
                  np.float32)
    in_maps = []
    for c in range(N_CORES):
        in_maps.append({
            "x16": x16,
            "idx_img": per_core[c]["idx"],
            "dl_img": per_core[c]["dl"],
            "dinv_img": per_core[c]["dinv"],
            "iota": np.ascontiguousarray(iota),
            "W1": np.ascontiguousarray(np.asarray(inputs["W1"], np.float32)),
            "b1": np.asarray(inputs["b1"], np.float32).reshape(HID, 1),
            "W2": np.ascontiguousarray(np.asarray(inputs["W2"], np.float32)),
            "b2": np.asarray(inputs["b2"], np.float32).reshape(OUT_D, 1),
            "Wh": np.ascontiguousarray(wh),
            "bh": bh,
        })
    return in_maps


def kernel(x, edge_index, W1, b1, W2, b2, Wd, bd, Wp, bp):
    from concourse import bass_utils

    T, per_core, dinv = preprocess(edge_index)
    nc = build_program(T)
    in_maps = make_in_maps(dict(x=x, W1=W1, b1=b1, W2=W2, b2=b2, Wd=Wd,
                                bd=bd, Wp=Wp, bp=bp), per_core, dinv)
    res = bass_utils.run_bass_kernel_spmd(nc, in_maps,
                                          core_ids=list(range(N_CORES)))
    dur = np.empty((N_NODES, 1), np.float32)
    pha = np.empty((N_NODES, 1), np.float32)
    for c in range(N_CORES):
        o = res.results[c]["out"]
        dur[c * PER_CORE:(c + 1) * PER_CORE, 0] = o[0]
        pha[c * PER_CORE:(c + 1) * PER_CORE, 0] = o[1]
    return dur, pha
